# revision 28
# baseline (speedup 1.0000x reference)
"""Multi-head causal attention (B=2, S=2048, D=1024, H=16, Dh=64) on 8
axon-tunneled TRN2 NeuronCores.

Sharding: core = b*4 + g handles batch b and head group g (4 heads, 256
feature columns of the QKV projections / 256 rows of Wo).  Each core is
fully independent; the host sums the 4 per-head-group partial outputs of
each batch.

v2 restructure: the kernel is ACT(exp)-bound (~87us of ACTIVATE stream),
so everything else is scheduled underneath the exp stream:
  - x is DMA'd in quarter-chunks; only Q(0,0), K(0,0), va[0..3] are
    emitted up front, so attention group (qc=0, pair=0) starts ~11us in.
  - All remaining projection work (Q/K chunks, V groups) and the output
    projections are queued as fine-grained closures ("filler") drained a
    few instructions per attention block - the PE executes them in the
    slack while ACT runs exp.
  - Diagonal-block exp is one wide ACTIVATE over [lo:1024] (the unwritten
    [512:512+lo] gap exps stale PSUM - finite, never read).  The causal
    mask multiply is one DVE op using a doubled [mask|mask] tile.
  - At each group end cps PSUM is staged to SBUF immediately (frees the
    single-buffered cps banks for the next group's PV) and the
    normalization (Ln/Exp reciprocal + PE partition-broadcast) is batched
    per qc, deferred one group, entirely off the critical path.
  - DMA issues (~600ns of issuing-engine time each) go to sync/gpsimd
    (+scalar only before the exp stream starts).
"""

import numpy as np
from collections import deque

D_IN = 1024
D_OUT = 1024
H = 16
DH = 64
B = 2
S = 2048
NCORES = 8
HG = 4            # heads per core
DG = HG * DH      # 256 feature cols per core

_state = {}


def _patch_tile_drain():
    """This image's walrus rejects instructions carrying >2 sync waits
    ("Too many sync wait commands"); Tile's final drain waits on every
    outstanding proc.  Split the waits into single-wait SP nops."""
    import concourse.tile as tile
    from concourse import mybir
    from concourse.vector_clock import ScopedClock

    if getattr(tile.TileContext._drain_and_barrier, "_split_waits", False):
        return

    def _drain_and_barrier(self, tick_clock, wait_clock):
        nc = self.nc
        probe = nc.sync.nop()
        wait_clock.add_sem_waits(
            probe.ins, ScopedClock({None: tick_clock.global_clock})
        )
        si = probe.ins.sync_info
        waits = list(si.on_wait) if si and si.on_wait else []
        if len(waits) > 1:
            probe.ins.sync_info = mybir.SyncInfo(
                on_wait=[waits[0]], on_update=list(si.on_update or [])
            )
            for w in waits[1:]:
                extra = nc.sync.nop()
                extra.ins.sync_info = mybir.SyncInfo(on_wait=[w], on_update=[])
        nc.sync.drain()

        nc.all_engine_barrier()
        assert self.sems is not None
        popped = nc._tile_sem_poison_stack.pop()
        assert popped is self._sem_poison
        nc.clear_and_free_semaphores(list(self.sems.allocated().values()))
        nc.all_engine_barrier()

    _drain_and_barrier._split_waits = True
    tile.TileContext._drain_and_barrier = _drain_and_barrier


def _split_excess_waits(nc, maxw=1):
    """Walrus in this image rejects instructions with too many sync-wait
    commands.  Hoist excess waits onto InstNoOp carriers inserted right
    before the offending instruction on the same engine (engines are
    in-order, so this preserves semantics)."""
    from concourse import mybir

    f = nc.m.functions[0]
    for bb in f.blocks:
        insts = bb.instructions  # live list
        i = 0
        while i < len(insts):
            ins = insts[i]
            si = ins.sync_info
            waits = list(si.on_wait) if si and si.on_wait else []
            if len(waits) > maxw:
                excess, keep = waits[:-maxw], waits[-maxw:]
                nops = []
                for j in range(0, len(excess), maxw):
                    nop = mybir.InstNoOp(
                        name=f"I-waitnop-{nc.next_id()}", ins=[], outs=[]
                    )
                    nop.engine = ins.engine
                    nop.sync_info = mybir.SyncInfo(
                        on_wait=excess[j : j + maxw], on_update=[]
                    )
                    nops.append(nop)
                ins.sync_info = mybir.SyncInfo(
                    on_wait=keep, on_update=list(si.on_update or [])
                )
                insts[i:i] = nops
                i += len(nops)
            i += 1


# va column layout: even heads [V|1] (65 cols), odd heads
# [1 | zeros*63 | V] (128 cols - the ones column at position 0 puts the
# odd head's softmax denominator at out partition 0, its ctx at 64:128).
VA_OFF = [0, 65, 193, 258]
VA_COLS = 386


def _build_nc():
    import concourse.bass as bass
    import concourse.tile as tile
    from concourse import mybir

    _patch_tile_drain()
    FP = mybir.dt.float32
    Alu = mybir.AluOpType
    Act = mybir.ActivationFunctionType
    MD = mybir.dt.bfloat16

    nc = bass.Bass("TRN2", target_bir_lowering=False, debug=False)
    d_xT = nc.dram_tensor("xT", [8, 128, S], MD, kind="ExternalInput").ap()
    d_wq = nc.dram_tensor("wq", [8, 128, DG], MD, kind="ExternalInput").ap()
    d_wk = nc.dram_tensor("wk", [8, 128, DG], MD, kind="ExternalInput").ap()
    d_wv = nc.dram_tensor("wv", [8, 128, DG], MD, kind="ExternalInput").ap()
    d_wo = nc.dram_tensor("wo", [2, 128, D_OUT], MD, kind="ExternalInput").ap()
    d_b = nc.dram_tensor("b6", [128, 6], FP, kind="ExternalInput").ap()
    # single causal-mask tile: mask for diagonal block j is the slice
    # E[:, 384-128j+lo : 896-128j]  (E[k,c] = c >= 384+k)
    d_mask = nc.dram_tensor("maskE", [128, 896], MD, kind="ExternalInput").ap()
    d_out = nc.dram_tensor("out", [S, D_OUT], MD, kind="ExternalOutput").ap()

    with tile.TileContext(nc) as tc:
        from contextlib import ExitStack

        with ExitStack() as ctx:
            const = ctx.enter_context(tc.tile_pool(name="const", bufs=1))
            qkv = ctx.enter_context(tc.tile_pool(name="qkv", bufs=1))
            scp = ctx.enter_context(tc.tile_pool(name="scp", bufs=2))
            normp = ctx.enter_context(tc.tile_pool(name="norm", bufs=2))
            osb = ctx.enter_context(tc.tile_pool(name="osb", bufs=3))
            ptp = ctx.enter_context(tc.tile_pool(name="pt", bufs=4))
            sp = ctx.enter_context(tc.tile_pool(name="spsum", bufs=2, space="PSUM"))
            cp = ctx.enter_context(tc.tile_pool(name="cpsum", bufs=1, space="PSUM"))
            wkp = ctx.enter_context(tc.tile_pool(name="wkpsum", bufs=2, space="PSUM"))

            wq_sb = [const.tile([128, DG], MD, tag=f"wq{i}", name=f"wq{i}") for i in range(8)]
            wk_sb = [const.tile([128, DG], MD, tag=f"wk{i}", name=f"wk{i}") for i in range(8)]
            wv_sb = [const.tile([128, DG], MD, tag=f"wv{i}", name=f"wv{i}") for i in range(8)]
            wo_sb = [const.tile([128, D_OUT], MD, tag=f"wo{i}", name=f"wo{i}") for i in range(2)]
            b_sb = const.tile([128, 6], FP, tag="b6", name="b6")
            mask_sb = const.tile([128, 896], MD, tag="maskE", name="maskE")
            # broadcast lhsT rows of ones at base partitions 0 and 32 (the
            # matmul base-partition legality set), bf16
            ones_a = const.tile([33, 128], MD, tag="onesa")
            nc.vector.memset(ones_a[0:1, :], 1.0)
            nc.vector.memset(ones_a[32:33, :], 1.0)

            xsb = [qkv.tile([128, S], MD, tag=f"x{i}", name=f"x{i}") for i in range(8)]
            qT = [qkv.tile([128, S], MD, tag=f"qT{i}", name=f"qT{i}") for i in range(2)]
            kT = [qkv.tile([128, S], MD, tag=f"kT{i}", name=f"kT{i}") for i in range(2)]
            va = [qkv.tile([128, VA_COLS], MD, tag=f"va{i}", name=f"va{i}") for i in range(16)]
            ctxT = [qkv.tile([128, S], MD, tag=f"ctxT{i}", name=f"ctxT{i}") for i in range(2)]

            # ---------------- DMA schedule ----------------
            # 3 queues while ACT is still idle (start-up), 2 after.
            q3 = [nc.sync, nc.scalar, nc.gpsimd]
            q2 = [nc.sync, nc.gpsimd]
            ri = [0]

            def dma3(dst, src):
                q3[ri[0] % 3].dma_start(dst, src)
                ri[0] += 1

            def dma2(dst, src):
                q2[ri[0] % 2].dma_start(dst, src)
                ri[0] += 1

            dma3(b_sb[:], d_b)
            for i in range(8):
                dma3(xsb[i][:, 0:512], d_xT[i][:, 0:512])
                dma3(wq_sb[i][:], d_wq[i])
                dma3(wk_sb[i][:], d_wk[i])
            for i in range(8):
                dma3(wv_sb[i][:], d_wv[i])
            dma3(mask_sb[:], d_mask)
            for i in range(8):
                dma3(xsb[i][:, 512:2048], d_xT[i][:, 512:2048])
            for i in range(2):
                dma3(wo_sb[i][:], d_wo[i])
            # va ones columns: even head h ones col at VA_OFF[h]+64, odd head
            # h+1 ones col at VA_OFF[h+1] - adjacent pairs (64,65) and
            # (257,258), so two memsets per tile.  The odd heads' pad columns
            # are left as stale SBUF: they only feed PSUM partitions 1:63 of
            # cps1, which nothing ever reads.
            for st in range(16):
                nc.vector.memset(va[st][:, 64:66], 1.0)
                nc.vector.memset(va[st][:, 257:259], 1.0)

            # ---------------- filler work units ----------------
            work = deque()

            def unit_qk(w_sb, bcol, dest, m, nq):
                """Q or K projection chunk: dest[m][:, nq*512:...] (9 closures)."""
                cell = {}
                sq = slice(nq * 512, (nq + 1) * 512)
                ms = slice(m * 128, (m + 1) * 128)
                cls = []

                def first():
                    cell["ps"] = wkp.tile([128, 512], FP, tag="w", name=f"qk{m}{nq}")
                    nc.tensor.matmul(
                        cell["ps"][:], w_sb[0][:, ms], xsb[0][:, sq],
                        start=True, stop=False,
                    )
                cls.append(first)
                for ci in range(1, 8):
                    def mid(ci=ci):
                        nc.tensor.matmul(
                            cell["ps"][:], w_sb[ci][:, ms], xsb[ci][:, sq],
                            start=False, stop=(ci == 7),
                        )
                    cls.append(mid)

                def last():
                    nc.vector.tensor_scalar(
                        dest[:, sq], cell["ps"][:], b_sb[:, bcol : bcol + 1],
                        None, Alu.add,
                    )
                cls.append(last)
                return cls

            def unit_va(st):
                """V projection for s-tile st -> va[st] (12 closures)."""
                cell = {}
                ss = slice(st * 128, (st + 1) * 128)
                cls = []

                def first():
                    cell["ps"] = wkp.tile([128, 512], FP, tag="w", name=f"pv{st}")
                    nc.tensor.matmul(
                        cell["ps"][:, 0:DG], xsb[0][:, ss], wv_sb[0][:],
                        start=True, stop=False,
                    )
                cls.append(first)
                for ci in range(1, 8):
                    def mid(ci=ci):
                        nc.tensor.matmul(
                            cell["ps"][:, 0:DG], xsb[ci][:, ss], wv_sb[ci][:],
                            start=False, stop=(ci == 7),
                        )
                    cls.append(mid)
                for h in range(HG):
                    def cast(h=h):
                        dst0 = VA_OFF[h] + (0 if h % 2 == 0 else 64)
                        nc.vector.tensor_copy(
                            va[st][:, dst0 : dst0 + 64],
                            cell["ps"][:, h * 64 : (h + 1) * 64],
                        )
                    cls.append(cast)
                return cls

            def unit_outproj(qc):
                """Output projection for q-chunk qc (28 closures)."""
                cls = []
                for st in range(4 * qc, 4 * qc + 4):
                    ss = slice(st * 128, (st + 1) * 128)
                    cell = {}

                    def alloc_ot(cell=cell, st=st):
                        cell["ot"] = osb.tile([128, 1024], MD, tag="ot", name=f"ot{st}")
                    cls.append(alloc_ot)
                    for n in range(2):
                        ns = slice(n * 512, (n + 1) * 512)
                        def mm0(cell=cell, ss=ss, ns=ns):
                            cell["ps"] = wkp.tile([128, 512], FP, tag="w", name="o")
                            nc.tensor.matmul(
                                cell["ps"][:], ctxT[0][:, ss], wo_sb[0][:, ns],
                                start=True, stop=False,
                            )
                        def mm1(cell=cell, ss=ss, ns=ns):
                            nc.tensor.matmul(
                                cell["ps"][:], ctxT[1][:, ss], wo_sb[1][:, ns],
                                start=False, stop=True,
                            )
                        def cast(cell=cell, ns=ns):
                            nc.vector.tensor_copy(cell["ot"][:, ns], cell["ps"][:])
                        cls.extend([mm0, mm1, cast])

                    if qc == 3:
                        # tail: split across queues so the last tiles drain
                        # in parallel
                        def dmaout(cell=cell, ss=ss):
                            s2 = slice(ss.start, ss.start + 64)
                            s3 = slice(ss.start + 64, ss.stop)
                            q3[ri[0] % 3].dma_start(d_out[s2, :], cell["ot"][0:64, :])
                            ri[0] += 1
                            q3[ri[0] % 3].dma_start(d_out[s3, :], cell["ot"][64:128, :])
                            ri[0] += 1
                    else:
                        def dmaout(cell=cell, ss=ss):
                            q3[ri[0] % 3].dma_start(d_out[ss, :], cell["ot"][:])
                            ri[0] += 1
                    cls.append(dmaout)
                return cls

            def run_all(cls):
                for c in cls:
                    c()

            # pre-loop: minimum for group (qc=0, ht=0)
            run_all(unit_qk(wq_sb, 0, qT[0], 0, 0))
            run_all(unit_qk(wk_sb, 2, kT[0], 0, 0))
            for st in range(4):
                run_all(unit_va(st))

            # filler queue in dependency order, with hard milestones: all
            # closures a group's inputs depend on MUST be emitted (popped)
            # before that group's first instruction is traced
            milestones = {}
            work.extend(unit_qk(wq_sb, 1, qT[1], 1, 0))
            work.extend(unit_qk(wk_sb, 3, kT[1], 1, 0))
            milestones[(0, 1)] = len(work)
            work.extend(unit_qk(wk_sb, 2, kT[0], 0, 1))
            work.extend(unit_qk(wq_sb, 0, qT[0], 0, 1))
            for st in range(4, 8):
                work.extend(unit_va(st))
            milestones[(1, 0)] = len(work)
            work.extend(unit_qk(wk_sb, 3, kT[1], 1, 1))
            work.extend(unit_qk(wq_sb, 1, qT[1], 1, 1))
            milestones[(1, 1)] = len(work)
            work.extend(unit_qk(wk_sb, 2, kT[0], 0, 2))
            work.extend(unit_qk(wq_sb, 0, qT[0], 0, 2))
            for st in range(8, 12):
                work.extend(unit_va(st))
            milestones[(2, 0)] = len(work)
            work.extend(unit_qk(wk_sb, 3, kT[1], 1, 2))
            work.extend(unit_qk(wq_sb, 1, qT[1], 1, 2))
            milestones[(2, 1)] = len(work)
            work.extend(unit_qk(wk_sb, 2, kT[0], 0, 3))
            work.extend(unit_qk(wq_sb, 0, qT[0], 0, 3))
            for st in range(12, 16):
                work.extend(unit_va(st))
            milestones[(3, 0)] = len(work)
            work.extend(unit_qk(wk_sb, 3, kT[1], 1, 3))
            work.extend(unit_qk(wq_sb, 1, qT[1], 1, 3))
            milestones[(3, 1)] = len(work)

            work2 = deque()  # outproj closures, appended at runtime
            GROUP_ORDER = [(qc, ht) for qc in range(4) for ht in range(2)]
            TOTAL_WORK = len(work)
            popped = [0]
            cur_quota = [2]

            def pop1():
                work.popleft()()
                popped[0] += 1

            def drain():
                if work2:
                    work2.popleft()()
                    if work2:
                        work2.popleft()()
                n = min(len(work), cur_quota[0])
                for _ in range(n):
                    pop1()

            def start_group(qc, ht):
                # everything earlier groups depend on must already be traced
                need = milestones.get((qc, ht), 0)
                while popped[0] < need:
                    pop1()
                # spread the next milestone's work evenly over this group
                gi = GROUP_ORDER.index((qc, ht))
                target = (
                    milestones[GROUP_ORDER[gi + 1]]
                    if gi + 1 < len(GROUP_ORDER)
                    else TOTAL_WORK
                )
                nkb = 4 * qc + 4
                cur_quota[0] = max(1, -(-(target - popped[0]) // nkb))

            # ---------------- normalization ----------------
            # sc staging per group (frees cps); recip (Ln/Exp) and apply
            # (broadcast matmul + scale) are emitted at different points so
            # the bc matmuls never sit in the PE queue waiting on ACT.
            sc_tiles = {}  # (qc, ht) -> (sc0, sc1)
            rec_tiles = {}  # (qc, ht) -> (rec, col_slice)

            def stage_group(qc, ht, cps0, cps1):
                a = "a" if ht == 0 else "b"
                sc0 = scp.tile([65, 512], FP, tag=f"{a}0", name=f"sc{a}0")
                sc1 = scp.tile([128, 512], FP, tag=f"{a}1", name=f"sc{a}1")
                nc.vector.tensor_copy(sc0[:, :], cps0[:, :])
                nc.vector.tensor_copy(sc1[:, :], cps1[:, :])
                sc_tiles[(qc, ht)] = (sc0, sc1)

            def norm_recip(groups):
                """Ln/Exp reciprocal of the denominators of `groups`.
                Denominators live at rows {0 (even head), 32 (odd head)} x
                one 512-col band per group of a [33, 512*len] tile so the bc
                matmul base partitions stay in the legal {0,32,64} set."""
                w = 512 * len(groups)
                dd = normp.tile([33, w], FP, tag="dd", name="dd")
                rec0 = normp.tile([33, w], FP, tag="rec0", name="rec0")
                rec = normp.tile([33, w], MD, tag="rec", name="rec")
                for i, g in enumerate(groups):
                    sc0, sc1 = sc_tiles[g]
                    cs = slice(i * 512, i * 512 + 512)
                    nc.vector.tensor_copy(dd[0:1, cs], sc0[64:65, :])
                    nc.vector.tensor_copy(dd[32:33, cs], sc1[0:1, :])
                    rec_tiles[g] = (rec, cs)
                nc.scalar.activation(rec0[:, 0:w], dd[:, 0:w], Act.Ln)
                nc.scalar.activation(rec[:, 0:w], rec0[:, 0:w], Act.Exp, scale=-1.0)

            def norm_apply(groups):
                for g in groups:
                    qc, ht = g
                    qsl = slice(qc * 512, (qc + 1) * 512)
                    sc0, sc1 = sc_tiles.pop(g)
                    rec, cs = rec_tiles.pop(g)
                    bc = wkp.tile([128, 512], FP, tag="w", name=f"bc{ht}")
                    # odd head: full 128-partition broadcast, then even head
                    # overwrites partitions 0:64
                    nc.tensor.matmul(
                        bc[:, :], ones_a[32:33, :], rec[32:33, cs],
                        start=True, stop=True,
                    )
                    nc.tensor.matmul(
                        bc[0:64, :], ones_a[0:1, 0:64], rec[0:1, cs],
                        start=True, stop=True,
                    )
                    tmp = normp.tile([128, 512], FP, tag="tmp", name=f"tmp{ht}")
                    nc.vector.tensor_mul(tmp[0:64, :], sc0[0:64, :], bc[0:64, :])
                    nc.vector.tensor_mul(tmp[64:128, :], sc1[64:128, :], bc[64:128, :])
                    nc.vector.tensor_scalar(
                        ctxT[ht][:, qsl], tmp[:, :], b_sb[:, 4 + ht : 5 + ht],
                        None, Alu.add,
                    )

            # ------- attention: exp-paced, projections as filler -------
            for qc in range(4):
                qs0 = qc * 512
                for ht in range(2):
                    start_group(qc, ht)
                    nkb = 4 * qc + 4
                    cps0 = cp.tile([65, 512], FP, tag="c0", name="c0")
                    cps1 = cp.tile([128, 512], FP, tag="c1", name="c1")
                    prev = None
                    for kb in range(nkb):
                        ks = slice(kb * 128, (kb + 1) * 128)
                        j = kb - 4 * qc
                        lo = 128 * j if j > 0 else 0
                        qsl = slice(qs0 + lo, qs0 + 512)
                        sps = sp.tile([128, 1024], FP, tag="s", name="s")
                        for hp in range(2):
                            hs = slice(hp * 64, hp * 64 + 64)
                            nc.tensor.matmul(
                                sps[:, 512 * hp + lo : 512 * (hp + 1)],
                                kT[ht][hs, ks], qT[ht][hs, qsl],
                                start=True, stop=True,
                            )
                        if ht == 0 and qc > 0:
                            if kb == 1:
                                norm_recip([(qc - 1, 0), (qc - 1, 1)])
                            elif kb == 3:
                                norm_apply([(qc - 1, 0), (qc - 1, 1)])
                            elif kb == 4:
                                work2.extend(unit_outproj(qc - 1))
                        elif ht == 1 and qc == 3:
                            # last q-chunk: per-group norm so the kernel tail
                            # only waits on (3,1)'s chain
                            if kb == 1:
                                norm_recip([(3, 0)])
                            elif kb == 3:
                                norm_apply([(3, 0)])
                        pt = ptp.tile([128, 1024], MD, tag="pt", name="pt")
                        # one wide exp; the unwritten [512:512+lo] gap exps
                        # stale PSUM (finite) that nothing ever reads
                        nc.scalar.activation(
                            pt[:, lo:1024], sps[:, lo:1024], Act.Exp
                        )
                        if j >= 0:
                            # zero the causal triangle post-exp with slices of
                            # the binary bf16 mask tile (one per head half)
                            msl = slice(384 - 128 * j + lo, 896 - 128 * j)
                            nc.vector.tensor_mul(
                                pt[:, lo:512], pt[:, lo:512], mask_sb[:, msl]
                            )
                            nc.vector.tensor_mul(
                                pt[:, 512 + lo : 1024], pt[:, 512 + lo : 1024],
                                mask_sb[:, msl],
                            )
                        if prev is not None:
                            pkb, plo, ppt = prev
                            h0, h1 = 2 * ht, 2 * ht + 1
                            nc.tensor.matmul(
                                cps0[:, plo:],
                                va[pkb][:, VA_OFF[h0] : VA_OFF[h0] + 65],
                                ppt[:, plo:512],
                                start=(pkb == 0), stop=(pkb == nkb - 1),
                            )
                            nc.tensor.matmul(
                                cps1[:, plo:],
                                va[pkb][:, VA_OFF[h1] : VA_OFF[h1] + 128],
                                ppt[:, 512 + plo : 1024],
                                start=(pkb == 0), stop=(pkb == nkb - 1),
                            )
                        prev = (kb, lo, pt)
                        drain()
                    # drain the last block's PV pair
                    pkb, plo, ppt = prev
                    h0, h1 = 2 * ht, 2 * ht + 1
                    nc.tensor.matmul(
                        cps0[:, plo:],
                        va[pkb][:, VA_OFF[h0] : VA_OFF[h0] + 65],
                        ppt[:, plo:512],
                        start=(pkb == 0), stop=(pkb == nkb - 1),
                    )
                    nc.tensor.matmul(
                        cps1[:, plo:],
                        va[pkb][:, VA_OFF[h1] : VA_OFF[h1] + 128],
                        ppt[:, 512 + plo : 1024],
                        start=(pkb == 0), stop=(pkb == nkb - 1),
                    )
                    stage_group(qc, ht, cps0, cps1)

            while work:
                work.popleft()()
            while work2:
                work2.popleft()()
            norm_recip([(3, 1)])
            norm_apply([(3, 1)])
            for c in unit_outproj(3):
                c()

    _split_excess_waits(nc)
    return nc


def _get_nc():
    if "nc" not in _state:
        _state["nc"] = _build_nc()
    return _state["nc"]


def _host_maskE():
    # E[k, c] = 1 if c >= 384 + k else 0; the mask for diagonal block j over
    # q-cols [lo..512) is the slice E[:, 384-128j+lo : 896-128j]
    k = np.arange(128)[:, None]
    c = np.arange(896)[None, :]
    return (c >= 384 + k).astype(np.float32)


def _build_in_maps(x, Wq, bq, Wk, bk, Wv, bv, Wo):
    import ml_dtypes

    md = ml_dtypes.bfloat16

    x = np.asarray(x, np.float32)
    Wq = np.asarray(Wq, np.float32)
    bq = np.asarray(bq, np.float32)
    Wk = np.asarray(Wk, np.float32)
    bk = np.asarray(bk, np.float32)
    Wv = np.asarray(Wv, np.float32)
    bv = np.asarray(bv, np.float32)
    Wo = np.asarray(Wo, np.float32)

    maskE = _host_maskE().astype(md)

    in_maps = []
    for core in range(NCORES):
        b, g = core // HG, core % HG
        cs = slice(g * DG, (g + 1) * DG)
        xT = np.ascontiguousarray(x[b].T).reshape(8, 128, S).astype(md)
        b6 = np.stack(
            [
                (0.125 * bq[cs]).reshape(2, 128)[0],
                (0.125 * bq[cs]).reshape(2, 128)[1],
                bk[cs].reshape(2, 128)[0],
                bk[cs].reshape(2, 128)[1],
                bv[cs].reshape(2, 128)[0],
                bv[cs].reshape(2, 128)[1],
            ],
            axis=1,
        ).astype(np.float32)
        in_maps.append(
            {
                "xT": xT,
                "wq": np.ascontiguousarray(0.125 * Wq[:, cs]).reshape(8, 128, DG).astype(md),
                "wk": np.ascontiguousarray(Wk[:, cs]).reshape(8, 128, DG).astype(md),
                "wv": np.ascontiguousarray(Wv[:, cs]).reshape(8, 128, DG).astype(md),
                "wo": np.ascontiguousarray(Wo[cs, :]).reshape(2, 128, D_OUT).astype(md),
                "b6": b6,
                "maskE": maskE,
            }
        )
    return in_maps


def kernel(x, Wq, bq, Wk, bk, Wv, bv, Wo):
    from concourse.bass_utils import run_bass_kernel_spmd

    nc = _get_nc()
    in_maps = _build_in_maps(x, Wq, bq, Wk, bk, Wv, bv, Wo)
    _state["in_maps"] = in_maps

    res = run_bass_kernel_spmd(nc, in_maps, list(range(NCORES)))
    out = np.zeros((B, S, D_OUT), np.float64)
    for core in range(NCORES):
        out[core // HG] += np.asarray(res.results[core]["out"], np.float32)
    return out.astype(np.float32)
